# revision 22
# baseline (speedup 1.0000x reference)
"""DIEN (GRU + attention + AUGRU) Trainium2 kernel, v2.

Data-parallel over 8 NeuronCores (64 batch rows each). All recurrent state is
feature-major (D on partitions, batch on free dim).

Key structure vs v1:
  - hist arrives pre-transposed from the host (X^T layout) -> no on-device
    transposes; a second natural-layout copy (t on partitions) feeds the
    masked history-sum matmuls, spread through the GRU phase.
  - Both recurrences run as TWO independent half-batch chains (32 cols each),
    interleaved to hide the per-step dependency-chain latency. PSUM-touching
    elementwise ops run on DVE, SBUF-only ones on GpSimd (idle in v1; it has
    no PSUM port).
  - Attention MLP layer 0 is folded to 3 matmuls per chunk: the query-linear
    term is precomputed once per core (qcol) and broadcast via a selector
    matmul; (q - fact) weights are folded into the fact weights host-side.
  - Scores land 2-steps-per-column in a (128, 100) psum slab (even t on
    partitions 0-63, odd t on 64-127). Softmax uses an additive -1e9 mask,
    exp with fused accumulation, and a stacked-identity matmul that sums the
    two halves AND broadcasts the denominator in one matmul.
  - Attention weights for the AUGRU are extracted with ONE PE transpose plus
    one SBUF->SBUF DMA into a partition-0 row; per-chunk broadcast to 128
    partitions is a single ones-vector matmul.
  - PSUM is bank-granular (2KB): per-chunk gates live in ONE merged slab
    [r|z|pn|hn]; sigmoid outputs share one rotating bank; scp2+hist share a
    persistent bank.
  - Biases in this model instance are all zero; bias work is only emitted
    when the host detects nonzero biases.
"""

import sys

if "/opt/trn_rl_repo" not in sys.path:
    sys.path.insert(0, "/opt/trn_rl_repo")

from contextlib import ExitStack

import ml_dtypes
import numpy as np

import concourse.bacc as bacc
import concourse.mybir as mybir
import concourse.tile as tile
from concourse.bass_utils import run_bass_kernel_spmd
from concourse.masks import make_identity

f32 = mybir.dt.float32
bf16 = mybir.dt.bfloat16
AF = mybir.ActivationFunctionType
ALU = mybir.AluOpType
BF = ml_dtypes.bfloat16

NCORES = 8
B, T, D = 512, 200, 128
BL = B // NCORES          # 64 batch rows per core
HB = BL // 2              # 32: chain width (2 chains per core)
CG = 2                    # recurrence steps per psum chunk
NCG = T // CG             # 100
CH = 8                    # attention chunk: steps per chunk
NCHUNK = T // CH          # 25
W = CH * BL               # 512
TH = 128                  # t-split for natural-layout hist tiles
TL = T - TH               # 72


def _mm(nc, out, lhsT, rhs, start, stop, skip=False):
    nc.tensor.matmul(out, lhsT, rhs, start=start, stop=stop,
                     skip_group_check=skip)


def build_nc(nonzero_bias=False):
    nc = bacc.Bacc("TRN2", target_bir_lowering=False)

    # ---------------- DRAM parameters --------------------------------
    histTd = nc.declare_dram_parameter("histT", [D, T * BL], bf16, isOutput=False)
    histN0d = nc.declare_dram_parameter("histN0", [TH, BL * D], bf16, isOutput=False)
    histN1d = nc.declare_dram_parameter("histN1", [TL, BL * D], bf16, isOutput=False)
    mstTd = nc.declare_dram_parameter("mstT", [TH, BL], bf16, isOutput=False)
    mstT1d = nc.declare_dram_parameter("mstT1", [TL, BL], bf16, isOutput=False)
    m2addd = nc.declare_dram_parameter("m2add", [2 * BL, T // 2], f32, isOutput=False)
    itemTbd = nc.declare_dram_parameter("itemTb", [D, BL], bf16, isOutput=False)
    userTbd = nc.declare_dram_parameter("userTb", [D, BL], bf16, isOutput=False)
    WihTd = nc.declare_dram_parameter("WihT", [D, 4 * D], bf16, isOutput=False)
    WhhTd = nc.declare_dram_parameter("WhhT", [D, 4 * D], bf16, isOutput=False)
    W0aTd = nc.declare_dram_parameter("W0aT", [D, 80], bf16, isOutput=False)
    W0cTd = nc.declare_dram_parameter("W0cT", [D, 80], bf16, isOutput=False)
    W0qTd = nc.declare_dram_parameter("W0qT", [D, 80], bf16, isOutput=False)
    selQd = nc.declare_dram_parameter("selQ", [BL, W], bf16, isOutput=False)
    stack2d = nc.declare_dram_parameter("stack2", [2 * BL, 2 * BL], f32, isOutput=False)
    W1Td = nc.declare_dram_parameter("W1T", [80, 40], bf16, isOutput=False)
    W2Td = nc.declare_dram_parameter("W2T", [40, 1], bf16, isOutput=False)
    augWd = nc.declare_dram_parameter("augW", [D, 6 * D], bf16, isOutput=False)
    outWTd = nc.declare_dram_parameter("outWT", [D, 5], bf16, isOutput=False)
    if nonzero_bias:
        bGd = nc.declare_dram_parameter("bG", [D, 4], f32, isOutput=False)
        bAd = nc.declare_dram_parameter("bA", [D, 3], f32, isOutput=False)
        b0d = nc.declare_dram_parameter("b0", [80, 1], f32, isOutput=False)
        b1d = nc.declare_dram_parameter("b1", [40, 1], f32, isOutput=False)
        b2od = nc.declare_dram_parameter("b2o", [1, 1], f32, isOutput=False)
    outd = nc.declare_dram_parameter("out", [1, BL], f32, isOutput=True)

    SIG, TANH = AF.Sigmoid, AF.Tanh

    with tile.TileContext(nc) as tc, ExitStack() as ctx:
        big = ctx.enter_context(tc.tile_pool(name="big", bufs=1))
        wp = ctx.enter_context(tc.tile_pool(name="wp", bufs=1))
        sp = ctx.enter_context(tc.tile_pool(name="sp", bufs=1))

        XT = big.tile([D, T * BL], bf16)
        XT3 = XT.rearrange("p (t b) -> p t b", b=BL)
        gruT = big.tile([D, T * BL], bf16)
        gruT3 = gruT.rearrange("p (t b) -> p t b", b=BL)
        histN0 = big.tile([TH, BL * D], bf16)
        histN1 = big.tile([TL, BL * D], bf16)
        rowsAll = big.tile([1, T * BL], bf16)

        # ---------------- DMAs -------------------------------------------
        WihT = wp.tile([D, 4 * D], bf16)
        nc.sync.dma_start(WihT[:, :], WihTd[:, :])
        WhhT = wp.tile([D, 4 * D], bf16)
        nc.sync.dma_start(WhhT[:, :], WhhTd[:, :])
        W0aT = wp.tile([D, 80], bf16)
        nc.sync.dma_start(W0aT[:, :], W0aTd[:, :])
        W0cT = wp.tile([D, 80], bf16)
        nc.sync.dma_start(W0cT[:, :], W0cTd[:, :])
        W0qT = wp.tile([D, 80], bf16)
        nc.sync.dma_start(W0qT[:, :], W0qTd[:, :])
        selQ = wp.tile([BL, W], bf16)
        nc.sync.dma_start(selQ[:, :], selQd[:, :])
        stack2 = wp.tile([2 * BL, 2 * BL], f32)
        nc.sync.dma_start(stack2[:, :], stack2d[:, :])
        W1T = wp.tile([80, 40], bf16)
        nc.sync.dma_start(W1T[:, :], W1Td[:, :])
        W2T = wp.tile([40, 1], bf16)
        nc.sync.dma_start(W2T[:, :], W2Td[:, :])
        augW = wp.tile([D, 6 * D], bf16)
        nc.sync.dma_start(augW[:, :], augWd[:, :])
        outWT = wp.tile([D, 5], bf16)
        nc.sync.dma_start(outWT[:, :], outWTd[:, :])
        itemTb = wp.tile([D, BL], bf16)
        nc.sync.dma_start(itemTb[:, :], itemTbd[:, :])
        userTb = wp.tile([D, BL], bf16)
        nc.sync.dma_start(userTb[:, :], userTbd[:, :])
        mstT = wp.tile([TH, BL], bf16)
        nc.sync.dma_start(mstT[:, :], mstTd[:, :])
        mstT1 = wp.tile([TL, BL], bf16)
        nc.sync.dma_start(mstT1[:, :], mstT1d[:, :])
        m2add = wp.tile([2 * BL, T // 2], f32)
        nc.sync.dma_start(m2add[:, :], m2addd[:, :])
        if nonzero_bias:
            bG = wp.tile([D, 4], f32)
            nc.sync.dma_start(bG[:, :], bGd[:, :])
            bA = wp.tile([D, 3], f32)
            nc.sync.dma_start(bA[:, :], bAd[:, :])
            b0v = wp.tile([80, 1], f32)
            nc.sync.dma_start(b0v[:, :], b0d[:, :])
            b1v = wp.tile([40, 1], f32)
            nc.sync.dma_start(b1v[:, :], b1d[:, :])
            b2ov = wp.tile([1, 1], f32)
            nc.sync.dma_start(b2ov[:, :], b2od[:, :])

        WrhT, WrxT = augW[:, 0:D], augW[:, D:2 * D]
        WuhT, WuxT = augW[:, 2 * D:3 * D], augW[:, 3 * D:4 * D]
        WahT, WaxT = augW[:, 4 * D:5 * D], augW[:, 5 * D:6 * D]

        # X^T in t-ordered pieces so the GRU can start early (SP queue).
        XPIECE = [(0, 8), (8, 24), (24, 48), (48, 80), (80, 120), (120, 160),
                  (160, 200)]
        for (a, b) in XPIECE:
            nc.sync.dma_start(XT[:, a * BL:b * BL], histTd[:, a * BL:b * BL])
        # Natural-layout hist on the Activation hwdge queue (parallel).
        NPIECE = 4
        for g in range(NPIECE):
            s = slice(g * (BL // NPIECE) * D, (g + 1) * (BL // NPIECE) * D)
            nc.scalar.dma_start(histN0[:, s], histN0d[:, s])
        for g in range(NPIECE):
            s = slice(g * (BL // NPIECE) * D, (g + 1) * (BL // NPIECE) * D)
            nc.scalar.dma_start(histN1[:, s], histN1d[:, s])

        identB128 = sp.tile([D, D], bf16)
        make_identity(nc, identB128)
        ones1 = sp.tile([1, D], bf16)
        nc.vector.memset(ones1[:, :], 1.0)
        zeroT = sp.tile([D, BL], bf16)
        nc.vector.memset(zeroT[:, :], 0.0)

        qcolS = sp.tile([BL, 80], bf16)
        wgt2 = sp.tile([2 * BL, T // 2], bf16)
        histF = sp.tile([D, BL], f32)
        histTb = sp.tile([D, BL], bf16)

        # ================= GRU phase =====================================
        with tc.tile_pool(name="gip", bufs=2, space="PSUM") as gip, \
             tc.tile_pool(name="pers", bufs=1, space="PSUM") as pers, \
             tc.tile_pool(name="mp", bufs=1, space="PSUM") as mp, \
             tc.tile_pool(name="y0p_", bufs=1, space="PSUM") as y0pp, \
             tc.tile_pool(name="y1p_", bufs=1, space="PSUM") as y1pp, \
             tc.tile_pool(name="gt", bufs=3) as gt, \
             tc.tile_pool(name="at", bufs=2) as at:
            persist = pers.tile([D, 512], f32)
            scp2 = persist[:, 0:T // 2]
            hist0 = persist[:, 256:256 + BL]
            hist1 = persist[:, 256 + BL:256 + 2 * BL]
            misc = mp.tile([2 * BL, 512], f32)
            qcp = misc[0:BL, 0:80]
            ssum = misc[:, 128:129]

            _mm(nc, qcp, itemTb[:, :], W0qT[:, :], start=True, stop=True)
            nc.scalar.copy(qcolS[:, :], qcp)

            # pending small ops (attention pieces / hist matmuls) popped one
            # per recurrence step so no single op blocks an engine queue long
            pend = []

            def pop_pending(k=2):
                for _ in range(min(k, len(pend))):
                    pend.pop(0)()

            def attention_chunk(c, spacers=()):
                gc = gruT[:, c * W:(c + 1) * W]
                r2 = at.tile([D, W], bf16, tag="r2")
                y0 = at.tile([80, W], bf16, tag="y0s")
                y1 = at.tile([40, W], bf16, tag="y1s")
                y0p = y0pp.tile([80, W], f32, tag="y0")
                y1p = y1pp.tile([40, W], f32, tag="y1")

                def mk_r2(j):
                    def f():
                        nc.gpsimd.tensor_mul(
                            r2[:, :].rearrange("p (s c) -> p s c", s=CH)
                            [:, 4 * j:4 * (j + 1), :],
                            gc.rearrange("p (s c) -> p s c", s=CH)
                            [:, 4 * j:4 * (j + 1), :],
                            itemTb[:, :].unsqueeze(1)
                            .broadcast_to([D, 4, BL]))
                    return f

                def mk_mm(i):
                    def f():
                        if i == 0:
                            _mm(nc, y0p[:, :], W0aT[:, :], gc,
                                start=True, stop=False)
                        elif i == 1:
                            _mm(nc, y0p[:, :], W0cT[:, :], r2[:, :],
                                start=False, stop=False)
                        else:
                            _mm(nc, y0p[:, :], qcolS[:, :], selQ[:, :],
                                start=False, stop=True)
                    return f

                def mk_relu0(j):
                    def f():
                        nc.scalar.activation(
                            y0[:, j * 128:(j + 1) * 128],
                            y0p[:, j * 128:(j + 1) * 128], AF.Relu,
                            bias=(b0v[:, 0:1] if nonzero_bias else 0.0))
                    return f

                def mk_y1(j):
                    def f():
                        _mm(nc, y1p[:, j * 256:(j + 1) * 256], W1T[:, :],
                            y0[:, j * 256:(j + 1) * 256],
                            start=True, stop=True, skip=(j > 0))
                    return f

                def mk_relu1(j):
                    def f():
                        nc.scalar.activation(
                            y1[:, j * 128:(j + 1) * 128],
                            y1p[:, j * 128:(j + 1) * 128], AF.Relu,
                            bias=(b1v[:, 0:1] if nonzero_bias else 0.0))
                    return f

                def mk_score(j):
                    def f():
                        _mm(nc, scp2[:, c * (CH // 2) + j:
                                     c * (CH // 2) + j + 1],
                            y1[:, j * 2 * BL:(j + 1) * 2 * BL], W2T[:, :],
                            start=True, stop=True)
                    return f

                sp = list(spacers)
                def spc(k):
                    for _ in range(min(k, len(sp))):
                        pend.append(sp.pop(0))
                for j in range(2):
                    pend.append(mk_r2(j))
                spc(1)
                for i in range(3):
                    pend.append(mk_mm(i))
                spc(2)
                for j in range(4):
                    pend.append(mk_relu0(j))
                spc(1)
                for j in range(2):
                    pend.append(mk_y1(j))
                spc(1)
                for j in range(4):
                    pend.append(mk_relu1(j))
                spc(1)
                for j in range(CH // 2):
                    pend.append(mk_score(j))
                pend.extend(sp)

            def mk_hist(hh, b):
                def f():
                    if hh == 0:
                        _mm(nc, hist0[:, b:b + 1],
                            histN0[:, b * D:(b + 1) * D], mstT[:, b:b + 1],
                            start=True, stop=True)
                    else:
                        _mm(nc, hist1[:, b:b + 1],
                            histN1[:, b * D:(b + 1) * D], mstT1[:, b:b + 1],
                            start=True, stop=True)
                return f

            hist_jobs = [mk_hist(0, b) for b in range(BL)] + \
                        [mk_hist(1, b) for b in range(BL)]

            NG = 5  # slab gates: [r | z | omz | pn | hn]
            for c in range(NCG):
                t0 = c * CG
                XTc = XT[:, t0 * BL:(t0 + CG) * BL]
                slab = gip.tile([D, NG * CG * BL], f32, tag="slab")
                sl = slab.rearrange("p (g s c) -> p g s c", g=NG, c=BL)
                _mm(nc, sl[:, 0, :, :], WihT[:, 0:D], XTc,
                    start=True, stop=True)
                _mm(nc, sl[:, 1, :, :], WihT[:, D:2 * D], XTc,
                    start=True, stop=True, skip=True)
                _mm(nc, sl[:, 2, :, :], WihT[:, 2 * D:3 * D], XTc,
                    start=True, stop=True, skip=True)
                _mm(nc, sl[:, 3, :, :], WihT[:, 3 * D:4 * D], XTc,
                    start=True, stop=True, skip=True)
                if nonzero_bias:
                    nc.vector.tensor_add(
                        sl[:, 0:2, :, :], sl[:, 0:2, :, :],
                        bG[:, 0:2].unsqueeze(2).unsqueeze(3)
                        .broadcast_to([D, 2, CG, BL]))
                    nc.vector.tensor_sub(
                        sl[:, 2, :, :], sl[:, 2, :, :],
                        bG[:, 1:2].unsqueeze(2).broadcast_to([D, CG, BL]))
                for s in range(CG):
                    t = t0 + s
                    hprev = (gruT3[:, t - 1, :] if t > 0 else zeroT[:, :])
                    _mm(nc, sl[:, 0, s, :], WhhT[:, 0:D], hprev,
                        start=False, stop=True, skip=True)
                    _mm(nc, sl[:, 1, s, :], WhhT[:, D:2 * D], hprev,
                        start=False, stop=True, skip=True)
                    _mm(nc, sl[:, 2, s, :], WhhT[:, 2 * D:3 * D], hprev,
                        start=False, stop=True, skip=True)
                    _mm(nc, sl[:, 4, s, :], WhhT[:, 3 * D:4 * D], hprev,
                        start=True, stop=True, skip=True)
                    pop_pending(3)
                    gS = gt.tile([D, 3 * BL], bf16, tag="gS")
                    gS3 = gS.rearrange("p (g c) -> p g c", g=3)
                    nc.scalar.activation(gS3[:, :, :], sl[:, 0:3, s, :], SIG)
                    rG = gS3[:, 0, :]
                    z, omz = gS3[:, 1, :], gS3[:, 2, :]
                    zh = gt.tile([D, BL], bf16, tag="zh")
                    nc.gpsimd.tensor_mul(zh[:, :], z, hprev)
                    tmp = gt.tile([D, BL], f32, tag="tmp")
                    if nonzero_bias:
                        nc.vector.scalar_tensor_tensor(
                            tmp[:, :], sl[:, 4, s, :], bG[:, 2:3], rG,
                            ALU.add, ALU.mult)
                    else:
                        nc.vector.tensor_mul(tmp[:, :], sl[:, 4, s, :],
                                             rG)
                    tmp2 = gt.tile([D, BL], f32, tag="tmp2")
                    nc.vector.tensor_add(tmp2[:, :], tmp[:, :],
                                         sl[:, 3, s, :])
                    n = gt.tile([D, BL], bf16, tag="n")
                    nc.scalar.activation(n[:, :], tmp2[:, :], TANH,
                                         bias=(bG[:, 3:4] if nonzero_bias
                                               else 0.0))
                    on = gt.tile([D, BL], bf16, tag="on")
                    nc.vector.tensor_mul(on[:, :], omz, n[:, :])
                    nc.vector.tensor_add(gruT3[:, t, :], on[:, :], zh[:, :])
                if c % 4 == 3:
                    ac = c // 4
                    if ac >= 1:
                        lo = (ac - 1) * 6
                        attention_chunk(ac, hist_jobs[lo:lo + 6])
                    else:
                        attention_chunk(ac)
            pend.extend(hist_jobs[24 * 6:])
            while pend:
                pop_pending(4)

            # ---------------- softmax over t ----------------------------
            rawm = at.tile([2 * BL, T // 2], f32, tag="rawm")
            if nonzero_bias:
                raw0 = at.tile([2 * BL, T // 2], f32, tag="raw0")
                nc.vector.tensor_scalar(raw0[:, :], scp2, b2ov[0:1, 0:1],
                                        0.0, ALU.add, ALU.max)
                nc.vector.tensor_add(rawm[:, :], raw0[:, :], m2add[:, :])
            else:
                nc.vector.scalar_tensor_tensor(rawm[:, :], scp2, 0.0,
                                               m2add[:, :], ALU.max, ALU.add)
            ex = at.tile([2 * BL, T // 2], f32, tag="ex")
            s1 = at.tile([2 * BL, 1], f32, tag="s1")
            nc.scalar.activation(ex[:, :], rawm[:, :], AF.Exp,
                                 accum_out=s1[:, :])
            _mm(nc, ssum, stack2[:, :], s1[:, :], start=True, stop=True)
            rs2 = at.tile([2 * BL, 1], f32, tag="rs2")
            nc.vector.reciprocal(rs2[:, :], ssum)
            nc.vector.tensor_scalar_mul(wgt2[:, :], ex[:, :], rs2[:, 0:1])

            # history mean: combine the two halves
            nc.scalar.copy(histF[:, :], hist0)
            nc.vector.tensor_add(histF[:, :], histF[:, :], hist1)
            nc.scalar.copy(histTb[:, :], histF[:, :])

            # attention weight rows: (128,100) -> (100,128) -> row layout
            wtp = misc[0:T // 2, 300:364].bitcast(bf16)
            nc.tensor.transpose(wtp, wgt2[:, :], identB128[:, :])
            rows_s = at.tile([T // 2, 2 * BL], bf16, tag="rows_s")
            nc.scalar.copy(rows_s[:, :], wtp)
            nc.sync.dma_start(rowsAll[:, :], rows_s[:, :])

        # ================= AUGRU =========================================
        hA = [zeroT[:, :]]
        with tc.tile_pool(name="axp", bufs=2, space="PSUM") as axp, \
             tc.tile_pool(name="abp", bufs=2, space="PSUM") as abp, \
             tc.tile_pool(name="ut", bufs=3) as ut:
            for c in range(NCG):
                t0 = c * CG
                gc = gruT[:, t0 * BL:(t0 + CG) * BL]
                # slab: [r | u | xh] x CG steps x 64
                slab = axp.tile([D, 3 * CG * BL], f32, tag="ru")
                sl4 = slab.rearrange("p (g s c) -> p g s c", g=3, c=BL)
                _mm(nc, sl4[:, 0, :, :], WrxT, gc, start=True, stop=True)
                _mm(nc, sl4[:, 1, :, :], WuxT, gc, start=True, stop=True,
                    skip=True)
                _mm(nc, sl4[:, 2, :, :], WaxT, gc, start=True, stop=True,
                    skip=True)
                pab = abp.tile([D, CG * BL], f32, tag="pab")
                pab3 = pab.rearrange("p (s c) -> p s c", c=BL)
                _mm(nc, pab[:, :], ones1[:, :],
                    rowsAll[:, t0 * BL:(t0 + CG) * BL], start=True, stop=True)
                if nonzero_bias:
                    nc.vector.tensor_add(
                        sl4[:, 0:2, :, :], sl4[:, 0:2, :, :],
                        bA[:, 0:2].unsqueeze(2).unsqueeze(3)
                        .broadcast_to([D, 2, CG, BL]))
                for s in range(CG):
                    t = t0 + s
                    hprev = hA[0]
                    _mm(nc, sl4[:, 0, s, :], WrhT, hprev,
                        start=False, stop=True, skip=True)
                    _mm(nc, sl4[:, 1, s, :], WuhT, hprev,
                        start=False, stop=True, skip=True)
                    ruS = ut.tile([D, 2 * BL], bf16, tag="ruS")
                    ruS2 = ruS.rearrange("p (g c) -> p g c", g=2)
                    nc.scalar.activation(ruS2[:, :, :], sl4[:, 0:2, s, :],
                                         SIG)
                    hr = ut.tile([D, BL], bf16, tag="hr")
                    nc.vector.tensor_mul(hr[:, :], hprev, ruS2[:, 0, :])
                    _mm(nc, sl4[:, 2, s, :], WahT, hr[:, :],
                        start=False, stop=True, skip=True)
                    # off-path: u' = a*u, 1-u', (1-u')*h
                    up = ut.tile([D, BL], bf16, tag="up")
                    nc.vector.tensor_mul(up[:, :], pab3[:, s, :],
                                         ruS2[:, 1, :])
                    omu = ut.tile([D, BL], bf16, tag="omu")
                    nc.gpsimd.tensor_scalar(omu[:, :], up[:, :], -1.0,
                                            1.0, ALU.mult, ALU.add)
                    omuh = ut.tile([D, BL], bf16, tag="omuh")
                    nc.gpsimd.tensor_mul(omuh[:, :], omu[:, :], hprev)
                    hh = ut.tile([D, BL], bf16, tag="hh")
                    if nonzero_bias:
                        nc.scalar.activation(hh[:, :], sl4[:, 2, s, :],
                                             TANH, bias=bA[:, 2:3])
                    else:
                        nc.scalar.activation(hh[:, :], sl4[:, 2, s, :], TANH)
                    uh = ut.tile([D, BL], bf16, tag="uh")
                    nc.vector.tensor_mul(uh[:, :], up[:, :], hh[:, :])
                    hnew = ut.tile([D, BL], bf16, tag="hA")
                    nc.vector.tensor_add(hnew[:, :], uh[:, :], omuh[:, :])
                    hA[0] = hnew

        # ---------------- output layer -----------------------------------
        with tc.tile_pool(name="ops", bufs=1, space="PSUM") as ops, \
             tc.tile_pool(name="ot", bufs=1) as ot:
            ih = ot.tile([D, BL], bf16)
            nc.vector.tensor_mul(ih[:, :], itemTb[:, :], histTb[:, :])
            po = ops.tile([1, BL], f32)
            pieces = [(userTb[:, :], 0), (itemTb[:, :], 1), (histTb[:, :], 2),
                      (ih[:, :], 3)]
            for piece, g in pieces:
                _mm(nc, po[:, :], outWT[:, g:g + 1], piece,
                    start=(g == 0), stop=False)
            _mm(nc, po[:, :], outWT[:, 4:5], hA[0],
                start=False, stop=True)
            outs = ot.tile([1, BL], f32)
            if nonzero_bias:
                nc.scalar.activation(outs[:, :], po[:, :], AF.Identity,
                                     bias=b2ov[0:1, 0:1])
            else:
                nc.scalar.copy(outs[:, :], po[:, :])
            nc.sync.dma_start(outd[:, :], outs[:, :])

    nc.finalize()
    return nc


_NC = {}


def _get_nc(nonzero_bias=False):
    if nonzero_bias not in _NC:
        _NC[nonzero_bias] = build_nc(nonzero_bias)
    return _NC[nonzero_bias]


def make_in_maps(inputs):
    """Slice + marshal full inputs into per-core input maps (host-side)."""
    f = {k: np.asarray(v) for k, v in inputs.items()}
    nonzero_bias = any(
        np.any(f[k]) for k in ("gru_bih", "gru_bhh", "attn_b0", "attn_b1",
                               "attn_b2", "aug_br", "aug_bu", "aug_bh",
                               "out_b"))

    # gate layout [r | z | -z | n]: the -z columns give omz = sigmoid(-x_z)
    # = 1 - z via the same merged sigmoid, at the cost of one extra matmul.
    Wih_t = f["gru_Wih"].T    # (128, 384) [r|z|n]
    Whh_t = f["gru_Whh"].T
    WihT = np.ascontiguousarray(np.concatenate(
        [Wih_t[:, 0:D], Wih_t[:, D:2 * D], -Wih_t[:, D:2 * D],
         Wih_t[:, 2 * D:3 * D]], axis=1).astype(BF))             # (128, 512)
    WhhT = np.ascontiguousarray(np.concatenate(
        [Whh_t[:, 0:D], Whh_t[:, D:2 * D], -Whh_t[:, D:2 * D],
         Whh_t[:, 2 * D:3 * D]], axis=1).astype(BF))
    W0 = f["attn_W0"]                                            # (80, 512)
    W0f, W0q, W0m, W0d = (W0[:, 0:D], W0[:, D:2 * D],
                          W0[:, 2 * D:3 * D], W0[:, 3 * D:4 * D])
    W0aT = np.ascontiguousarray((W0f - W0d).T.astype(BF))        # (128, 80)
    W0cT = np.ascontiguousarray(W0m.T.astype(BF))
    W0qT = np.ascontiguousarray((W0q + W0d).T.astype(BF))
    selQ = np.ascontiguousarray(np.tile(np.eye(BL, dtype=np.float32),
                                        (1, CH)).astype(BF))     # (64, 512)
    stack2 = np.ascontiguousarray(np.tile(np.eye(BL, dtype=np.float32),
                                          (2, 2)))               # (128, 128)
    W1T = np.ascontiguousarray(f["attn_W1"].T.astype(BF))        # (80, 40)
    W2T = np.ascontiguousarray(f["attn_W2"].T.astype(BF))        # (40, 1)
    augW = np.concatenate(
        [np.ascontiguousarray(f[k][:, p * D:(p + 1) * D].T)
         for k in ("aug_Wr", "aug_Wu", "aug_Wh") for p in (0, 1)],
        axis=1).astype(BF)                                       # (128, 768)
    outWT = np.ascontiguousarray(f["out_W"].reshape(5, D).T.astype(BF))

    shared = dict(WihT=WihT, WhhT=WhhT, W0aT=W0aT, W0cT=W0cT, W0qT=W0qT,
                  selQ=selQ, stack2=stack2, W1T=W1T, W2T=W2T, augW=augW,
                  outWT=outWT)
    if nonzero_bias:
        bih3, bhh3 = f["gru_bih"].reshape(3, D), f["gru_bhh"].reshape(3, D)
        # [br, bz, bhn, bin]: r/z biases merged; n-gate keeps the recurrent
        # part (scaled by r) and the input part (tanh bias) separate.
        bG = np.stack([bih3[0] + bhh3[0], bih3[1] + bhh3[1], bhh3[2],
                       bih3[2]], axis=1).astype(np.float32)      # (128, 4)
        bA = np.stack([f["aug_br"], f["aug_bu"], f["aug_bh"]],
                      axis=1).astype(np.float32)
        shared.update(
            bG=np.ascontiguousarray(bG), bA=np.ascontiguousarray(bA),
            b0=np.ascontiguousarray(f["attn_b0"].reshape(80, 1).astype(np.float32)),
            b1=np.ascontiguousarray(f["attn_b1"].reshape(40, 1).astype(np.float32)),
            b2o=np.ascontiguousarray(f["attn_b2"].reshape(1, 1).astype(np.float32)))

    mask = f["mask"].astype(np.float32)          # (B, T)
    hist = f["item_historical_embedding"]        # (B, T, D)
    in_maps = []
    for ci in range(NCORES):
        s = slice(ci * BL, (ci + 1) * BL)
        m = dict(shared)
        hs = hist[s]                                             # (64,200,128)
        m["histT"] = np.ascontiguousarray(
            hs.transpose(2, 1, 0).reshape(D, T * BL).astype(BF))
        m["histN0"] = np.ascontiguousarray(
            hs[:, 0:TH, :].transpose(1, 0, 2).reshape(TH, BL * D).astype(BF))
        m["histN1"] = np.ascontiguousarray(
            hs[:, TH:T, :].transpose(1, 0, 2).reshape(TL, BL * D).astype(BF))
        msk = mask[s]                                            # (64, 200)
        seq = f["sequential_length"][s].astype(np.float32)       # (64,)
        msc = msk / seq[:, None]                                 # (64, 200)
        m["mstT"] = np.ascontiguousarray(msc[:, 0:TH].T.astype(BF))
        m["mstT1"] = np.ascontiguousarray(msc[:, TH:T].T.astype(BF))
        # additive softmax mask in the (128, 100) 2-step layout:
        # row p<64 -> t even (t=2j, b=p); row p>=64 -> t odd (t=2j+1, b=p-64)
        m2 = np.zeros((2 * BL, T // 2), dtype=np.float32)
        m2[0:BL, :] = np.where(msk[:, 0::2] == 1, 0.0, -1e9)
        m2[BL:2 * BL, :] = np.where(msk[:, 1::2] == 1, 0.0, -1e9)
        m["m2add"] = np.ascontiguousarray(m2)
        m["itemTb"] = np.ascontiguousarray(f["item_embedding"][s].T.astype(BF))
        m["userTb"] = np.ascontiguousarray(f["user_embedding"][s].T.astype(BF))
        in_maps.append(m)
    return in_maps, nonzero_bias


def kernel(**inputs) -> np.ndarray:
    in_maps, nonzero_bias = make_in_maps(inputs)
    nc = _get_nc(nonzero_bias)
    res = run_bass_kernel_spmd(nc, in_maps, list(range(NCORES)))
    return np.concatenate(
        [np.asarray(res.results[c]["out"]).reshape(BL) for c in range(NCORES)])


# revision 24
# speedup vs baseline: 1.0070x; 1.0070x over previous
"""DIEN (GRU + attention + AUGRU) Trainium2 kernel, v2.

Data-parallel over 8 NeuronCores (64 batch rows each). All recurrent state is
feature-major (D on partitions, batch on free dim).

Key structure vs v1:
  - hist arrives pre-transposed from the host (X^T layout) -> no on-device
    transposes; a second natural-layout copy (t on partitions) feeds the
    masked history-sum matmuls, spread through the GRU phase.
  - Both recurrences run as TWO independent half-batch chains (32 cols each),
    interleaved to hide the per-step dependency-chain latency. PSUM-touching
    elementwise ops run on DVE, SBUF-only ones on GpSimd (idle in v1; it has
    no PSUM port).
  - Attention MLP layer 0 is folded to 3 matmuls per chunk: the query-linear
    term is precomputed once per core (qcol) and broadcast via a selector
    matmul; (q - fact) weights are folded into the fact weights host-side.
  - Scores land 2-steps-per-column in a (128, 100) psum slab (even t on
    partitions 0-63, odd t on 64-127). Softmax uses an additive -1e9 mask,
    exp with fused accumulation, and a stacked-identity matmul that sums the
    two halves AND broadcasts the denominator in one matmul.
  - Attention weights for the AUGRU are extracted with ONE PE transpose plus
    one SBUF->SBUF DMA into a partition-0 row; per-chunk broadcast to 128
    partitions is a single ones-vector matmul.
  - PSUM is bank-granular (2KB): per-chunk gates live in ONE merged slab
    [r|z|pn|hn]; sigmoid outputs share one rotating bank; scp2+hist share a
    persistent bank.
  - Biases in this model instance are all zero; bias work is only emitted
    when the host detects nonzero biases.
"""

import sys

if "/opt/trn_rl_repo" not in sys.path:
    sys.path.insert(0, "/opt/trn_rl_repo")

from contextlib import ExitStack

import ml_dtypes
import numpy as np

import concourse.bacc as bacc
import concourse.mybir as mybir
import concourse.tile as tile
from concourse.bass_utils import run_bass_kernel_spmd
from concourse.masks import make_identity

f32 = mybir.dt.float32
bf16 = mybir.dt.bfloat16
AF = mybir.ActivationFunctionType
ALU = mybir.AluOpType
BF = ml_dtypes.bfloat16

NCORES = 8
B, T, D = 512, 200, 128
BL = B // NCORES          # 64 batch rows per core
HB = BL // 2              # 32: chain width (2 chains per core)
CG = 2                    # recurrence steps per psum chunk
NCG = T // CG             # 100
CH = 8                    # attention chunk: steps per chunk
NCHUNK = T // CH          # 25
W = CH * BL               # 512
TH = 128                  # t-split for natural-layout hist tiles
TL = T - TH               # 72


def _mm(nc, out, lhsT, rhs, start, stop, skip=False):
    nc.tensor.matmul(out, lhsT, rhs, start=start, stop=stop,
                     skip_group_check=skip)


def build_nc(nonzero_bias=False):
    nc = bacc.Bacc("TRN2", target_bir_lowering=False)

    # ---------------- DRAM parameters --------------------------------
    histTd = nc.declare_dram_parameter("histT", [D, T * BL], bf16, isOutput=False)
    histN0d = nc.declare_dram_parameter("histN0", [TH, BL * D], bf16, isOutput=False)
    histN1d = nc.declare_dram_parameter("histN1", [TL, BL * D], bf16, isOutput=False)
    mstTd = nc.declare_dram_parameter("mstT", [TH, BL], bf16, isOutput=False)
    mstT1d = nc.declare_dram_parameter("mstT1", [TL, BL], bf16, isOutput=False)
    m2addd = nc.declare_dram_parameter("m2add", [2 * BL, T // 2], f32, isOutput=False)
    itemTbd = nc.declare_dram_parameter("itemTb", [D, BL], bf16, isOutput=False)
    userTbd = nc.declare_dram_parameter("userTb", [D, BL], bf16, isOutput=False)
    WihTd = nc.declare_dram_parameter("WihT", [D, 4 * D], bf16, isOutput=False)
    WhhTd = nc.declare_dram_parameter("WhhT", [D, 4 * D], bf16, isOutput=False)
    W0aTd = nc.declare_dram_parameter("W0aT", [D, 80], bf16, isOutput=False)
    W0cTd = nc.declare_dram_parameter("W0cT", [D, 80], bf16, isOutput=False)
    W0qTd = nc.declare_dram_parameter("W0qT", [D, 80], bf16, isOutput=False)
    selQd = nc.declare_dram_parameter("selQ", [BL, W], bf16, isOutput=False)
    stack2d = nc.declare_dram_parameter("stack2", [2 * BL, 2 * BL], f32, isOutput=False)
    W1Td = nc.declare_dram_parameter("W1T", [80, 40], bf16, isOutput=False)
    W2Td = nc.declare_dram_parameter("W2T", [40, 1], bf16, isOutput=False)
    augWd = nc.declare_dram_parameter("augW", [D, 6 * D], bf16, isOutput=False)
    outWTd = nc.declare_dram_parameter("outWT", [D, 5], bf16, isOutput=False)
    if nonzero_bias:
        bGd = nc.declare_dram_parameter("bG", [D, 4], f32, isOutput=False)
        bAd = nc.declare_dram_parameter("bA", [D, 3], f32, isOutput=False)
        b0d = nc.declare_dram_parameter("b0", [80, 1], f32, isOutput=False)
        b1d = nc.declare_dram_parameter("b1", [40, 1], f32, isOutput=False)
        b2od = nc.declare_dram_parameter("b2o", [1, 1], f32, isOutput=False)
    outd = nc.declare_dram_parameter("out", [1, BL], f32, isOutput=True)

    SIG, TANH = AF.Sigmoid, AF.Tanh

    with tile.TileContext(nc) as tc, ExitStack() as ctx:
        big = ctx.enter_context(tc.tile_pool(name="big", bufs=1))
        wp = ctx.enter_context(tc.tile_pool(name="wp", bufs=1))
        sp = ctx.enter_context(tc.tile_pool(name="sp", bufs=1))

        XT = big.tile([D, T * BL], bf16)
        XT3 = XT.rearrange("p (t b) -> p t b", b=BL)
        gruT = big.tile([D, T * BL], bf16)
        gruT3 = gruT.rearrange("p (t b) -> p t b", b=BL)
        histN0 = big.tile([TH, BL * D], bf16)
        histN1 = big.tile([TL, BL * D], bf16)
        rowsAll = big.tile([1, T * BL], bf16)

        # ---------------- DMAs -------------------------------------------
        WihT = wp.tile([D, 4 * D], bf16)
        nc.sync.dma_start(WihT[:, :], WihTd[:, :])
        WhhT = wp.tile([D, 4 * D], bf16)
        nc.sync.dma_start(WhhT[:, :], WhhTd[:, :])
        W0aT = wp.tile([D, 80], bf16)
        nc.sync.dma_start(W0aT[:, :], W0aTd[:, :])
        W0cT = wp.tile([D, 80], bf16)
        nc.sync.dma_start(W0cT[:, :], W0cTd[:, :])
        W0qT = wp.tile([D, 80], bf16)
        nc.sync.dma_start(W0qT[:, :], W0qTd[:, :])
        selQ = wp.tile([BL, W], bf16)
        nc.sync.dma_start(selQ[:, :], selQd[:, :])
        stack2 = wp.tile([2 * BL, 2 * BL], f32)
        nc.sync.dma_start(stack2[:, :], stack2d[:, :])
        W1T = wp.tile([80, 40], bf16)
        nc.sync.dma_start(W1T[:, :], W1Td[:, :])
        W2T = wp.tile([40, 1], bf16)
        nc.sync.dma_start(W2T[:, :], W2Td[:, :])
        augW = wp.tile([D, 6 * D], bf16)
        nc.sync.dma_start(augW[:, :], augWd[:, :])
        outWT = wp.tile([D, 5], bf16)
        nc.sync.dma_start(outWT[:, :], outWTd[:, :])
        itemTb = wp.tile([D, BL], bf16)
        nc.sync.dma_start(itemTb[:, :], itemTbd[:, :])
        userTb = wp.tile([D, BL], bf16)
        nc.sync.dma_start(userTb[:, :], userTbd[:, :])
        mstT = wp.tile([TH, BL], bf16)
        nc.sync.dma_start(mstT[:, :], mstTd[:, :])
        mstT1 = wp.tile([TL, BL], bf16)
        nc.sync.dma_start(mstT1[:, :], mstT1d[:, :])
        m2add = wp.tile([2 * BL, T // 2], f32)
        nc.sync.dma_start(m2add[:, :], m2addd[:, :])
        if nonzero_bias:
            bG = wp.tile([D, 4], f32)
            nc.sync.dma_start(bG[:, :], bGd[:, :])
            bA = wp.tile([D, 3], f32)
            nc.sync.dma_start(bA[:, :], bAd[:, :])
            b0v = wp.tile([80, 1], f32)
            nc.sync.dma_start(b0v[:, :], b0d[:, :])
            b1v = wp.tile([40, 1], f32)
            nc.sync.dma_start(b1v[:, :], b1d[:, :])
            b2ov = wp.tile([1, 1], f32)
            nc.sync.dma_start(b2ov[:, :], b2od[:, :])

        WrhT, WrxT = augW[:, 0:D], augW[:, D:2 * D]
        WuhT, WuxT = augW[:, 2 * D:3 * D], augW[:, 3 * D:4 * D]
        WahT, WaxT = augW[:, 4 * D:5 * D], augW[:, 5 * D:6 * D]

        # X^T in t-ordered pieces so the GRU can start early (SP queue).
        XPIECE = [(0, 8), (8, 24), (24, 48), (48, 80), (80, 120), (120, 160),
                  (160, 200)]
        for (a, b) in XPIECE:
            nc.sync.dma_start(XT[:, a * BL:b * BL], histTd[:, a * BL:b * BL])
        # Natural-layout hist on the Activation hwdge queue (parallel).
        NPIECE = 4
        for g in range(NPIECE):
            s = slice(g * (BL // NPIECE) * D, (g + 1) * (BL // NPIECE) * D)
            nc.scalar.dma_start(histN0[:, s], histN0d[:, s])
        for g in range(NPIECE):
            s = slice(g * (BL // NPIECE) * D, (g + 1) * (BL // NPIECE) * D)
            nc.scalar.dma_start(histN1[:, s], histN1d[:, s])

        identB128 = sp.tile([D, D], bf16)
        make_identity(nc, identB128)
        ones1 = sp.tile([1, D], bf16)
        nc.vector.memset(ones1[:, :], 1.0)
        zeroT = sp.tile([D, BL], bf16)
        nc.vector.memset(zeroT[:, :], 0.0)

        qcolS = sp.tile([BL, 80], bf16)
        wgt2 = sp.tile([2 * BL, T // 2], bf16)
        histF = sp.tile([D, BL], f32)
        histTb = sp.tile([D, BL], bf16)

        # ================= GRU phase =====================================
        with tc.tile_pool(name="gip", bufs=2, space="PSUM") as gip, \
             tc.tile_pool(name="pers", bufs=1, space="PSUM") as pers, \
             tc.tile_pool(name="mp", bufs=1, space="PSUM") as mp, \
             tc.tile_pool(name="y0p_", bufs=1, space="PSUM") as y0pp, \
             tc.tile_pool(name="y1p_", bufs=1, space="PSUM") as y1pp, \
             tc.tile_pool(name="gt", bufs=5) as gt, \
             tc.tile_pool(name="at", bufs=3) as at:
            persist = pers.tile([D, 512], f32)
            scp2 = persist[:, 0:T // 2]
            hist0 = persist[:, 256:256 + BL]
            hist1 = persist[:, 256 + BL:256 + 2 * BL]
            misc = mp.tile([2 * BL, 512], f32)
            qcp = misc[0:BL, 0:80]
            ssum = misc[:, 128:129]

            _mm(nc, qcp, itemTb[:, :], W0qT[:, :], start=True, stop=True)
            nc.scalar.copy(qcolS[:, :], qcp)

            # pending small ops (attention pieces / hist matmuls) popped one
            # per recurrence step so no single op blocks an engine queue long
            pend = []

            def pop_pending(k=2):
                for _ in range(min(k, len(pend))):
                    pend.pop(0)()

            def attention_chunk(c, spacers=()):
                gc = gruT[:, c * W:(c + 1) * W]
                r2 = at.tile([D, W], bf16, tag="r2")
                y0 = at.tile([80, W], bf16, tag="y0s")
                y1 = at.tile([40, W], bf16, tag="y1s")
                y0p = y0pp.tile([80, W], f32, tag="y0")
                y1p = y1pp.tile([40, W], f32, tag="y1")

                def mk_r2(j):
                    def f():
                        nc.vector.tensor_mul(
                            r2[:, :].rearrange("p (s c) -> p s c", s=CH)
                            [:, 4 * j:4 * (j + 1), :],
                            gc.rearrange("p (s c) -> p s c", s=CH)
                            [:, 4 * j:4 * (j + 1), :],
                            itemTb[:, :].unsqueeze(1)
                            .broadcast_to([D, 4, BL]))
                    return f

                def mk_mm(i):
                    def f():
                        if i == 0:
                            _mm(nc, y0p[:, :], W0aT[:, :], gc,
                                start=True, stop=False)
                        elif i == 1:
                            _mm(nc, y0p[:, :], W0cT[:, :], r2[:, :],
                                start=False, stop=False)
                        else:
                            _mm(nc, y0p[:, :], qcolS[:, :], selQ[:, :],
                                start=False, stop=True)
                    return f

                def mk_relu0(j):
                    def f():
                        nc.vector.tensor_scalar_max(
                            y0[:, j * 128:(j + 1) * 128],
                            y0p[:, j * 128:(j + 1) * 128], 0.0)
                    return f

                def mk_y1(j):
                    def f():
                        _mm(nc, y1p[:, j * 256:(j + 1) * 256], W1T[:, :],
                            y0[:, j * 256:(j + 1) * 256],
                            start=True, stop=True, skip=(j > 0))
                    return f

                def mk_relu1(j):
                    def f():
                        nc.vector.tensor_scalar_max(
                            y1[:, j * 128:(j + 1) * 128],
                            y1p[:, j * 128:(j + 1) * 128], 0.0)
                    return f

                def mk_score(j):
                    def f():
                        _mm(nc, scp2[:, c * (CH // 2) + j:
                                     c * (CH // 2) + j + 1],
                            y1[:, j * 2 * BL:(j + 1) * 2 * BL], W2T[:, :],
                            start=True, stop=True)
                    return f

                sp = list(spacers)
                def spc(k):
                    for _ in range(min(k, len(sp))):
                        pend.append(sp.pop(0))
                for j in range(2):
                    pend.append(mk_r2(j))
                spc(1)
                for i in range(3):
                    pend.append(mk_mm(i))
                spc(2)
                for j in range(4):
                    pend.append(mk_relu0(j))
                spc(1)
                for j in range(2):
                    pend.append(mk_y1(j))
                spc(1)
                for j in range(4):
                    pend.append(mk_relu1(j))
                spc(1)
                for j in range(CH // 2):
                    pend.append(mk_score(j))
                pend.extend(sp)

            def mk_hist(hh, b):
                def f():
                    if hh == 0:
                        _mm(nc, hist0[:, b:b + 1],
                            histN0[:, b * D:(b + 1) * D], mstT[:, b:b + 1],
                            start=True, stop=True)
                    else:
                        _mm(nc, hist1[:, b:b + 1],
                            histN1[:, b * D:(b + 1) * D], mstT1[:, b:b + 1],
                            start=True, stop=True)
                return f

            hist_jobs = [mk_hist(0, b) for b in range(BL)] + \
                        [mk_hist(1, b) for b in range(BL)]

            NG = 5  # slab gates: [r | z | omz | pn | hn]
            for c in range(NCG):
                t0 = c * CG
                XTc = XT[:, t0 * BL:(t0 + CG) * BL]
                slab = gip.tile([D, NG * CG * BL], f32, tag="slab")
                sl = slab.rearrange("p (g s c) -> p g s c", g=NG, c=BL)
                _mm(nc, sl[:, 0, :, :], WihT[:, 0:D], XTc,
                    start=True, stop=True)
                _mm(nc, sl[:, 1, :, :], WihT[:, D:2 * D], XTc,
                    start=True, stop=True, skip=True)
                _mm(nc, sl[:, 2, :, :], WihT[:, 2 * D:3 * D], XTc,
                    start=True, stop=True, skip=True)
                _mm(nc, sl[:, 3, :, :], WihT[:, 3 * D:4 * D], XTc,
                    start=True, stop=True, skip=True)
                if nonzero_bias:
                    nc.vector.tensor_add(
                        sl[:, 0:2, :, :], sl[:, 0:2, :, :],
                        bG[:, 0:2].unsqueeze(2).unsqueeze(3)
                        .broadcast_to([D, 2, CG, BL]))
                    nc.vector.tensor_sub(
                        sl[:, 2, :, :], sl[:, 2, :, :],
                        bG[:, 1:2].unsqueeze(2).broadcast_to([D, CG, BL]))
                for s in range(CG):
                    t = t0 + s
                    hprev = (gruT3[:, t - 1, :] if t > 0 else zeroT[:, :])
                    _mm(nc, sl[:, 0, s, :], WhhT[:, 0:D], hprev,
                        start=False, stop=True, skip=True)
                    _mm(nc, sl[:, 1, s, :], WhhT[:, D:2 * D], hprev,
                        start=False, stop=True, skip=True)
                    _mm(nc, sl[:, 2, s, :], WhhT[:, 2 * D:3 * D], hprev,
                        start=False, stop=True, skip=True)
                    _mm(nc, sl[:, 4, s, :], WhhT[:, 3 * D:4 * D], hprev,
                        start=True, stop=True, skip=True)
                    pop_pending(3)
                    gS = gt.tile([D, 3 * BL], bf16, tag="gS")
                    gS3 = gS.rearrange("p (g c) -> p g c", g=3)
                    nc.scalar.activation(gS3[:, :, :], sl[:, 0:3, s, :], SIG)
                    rG = gS3[:, 0, :]
                    z, omz = gS3[:, 1, :], gS3[:, 2, :]
                    zh = gt.tile([D, BL], bf16, tag="zh")
                    nc.gpsimd.tensor_mul(zh[:, :], z, hprev)
                    tmp = gt.tile([D, BL], f32, tag="tmp")
                    if nonzero_bias:
                        nc.vector.scalar_tensor_tensor(
                            tmp[:, :], sl[:, 4, s, :], bG[:, 2:3], rG,
                            ALU.add, ALU.mult)
                    else:
                        nc.vector.tensor_mul(tmp[:, :], sl[:, 4, s, :],
                                             rG)
                    tmp2 = gt.tile([D, BL], f32, tag="tmp2")
                    nc.vector.tensor_add(tmp2[:, :], tmp[:, :],
                                         sl[:, 3, s, :])
                    n = gt.tile([D, BL], bf16, tag="n")
                    nc.scalar.activation(n[:, :], tmp2[:, :], TANH,
                                         bias=(bG[:, 3:4] if nonzero_bias
                                               else 0.0))
                    on = gt.tile([D, BL], bf16, tag="on")
                    nc.vector.tensor_mul(on[:, :], omz, n[:, :])
                    nc.vector.tensor_add(gruT3[:, t, :], on[:, :], zh[:, :])
                if c % 4 == 3:
                    ac = c // 4
                    if ac >= 1:
                        lo = (ac - 1) * 6
                        attention_chunk(ac, hist_jobs[lo:lo + 6])
                    else:
                        attention_chunk(ac)
            pend.extend(hist_jobs[24 * 6:])
            while pend:
                pop_pending(4)

            # ---------------- softmax over t ----------------------------
            rawm = at.tile([2 * BL, T // 2], f32, tag="rawm")
            if nonzero_bias:
                raw0 = at.tile([2 * BL, T // 2], f32, tag="raw0")
                nc.vector.tensor_scalar(raw0[:, :], scp2, b2ov[0:1, 0:1],
                                        0.0, ALU.add, ALU.max)
                nc.vector.tensor_add(rawm[:, :], raw0[:, :], m2add[:, :])
            else:
                nc.vector.scalar_tensor_tensor(rawm[:, :], scp2, 0.0,
                                               m2add[:, :], ALU.max, ALU.add)
            ex = at.tile([2 * BL, T // 2], f32, tag="ex")
            s1 = at.tile([2 * BL, 1], f32, tag="s1")
            nc.scalar.activation(ex[:, :], rawm[:, :], AF.Exp,
                                 accum_out=s1[:, :])
            _mm(nc, ssum, stack2[:, :], s1[:, :], start=True, stop=True)
            rs2 = at.tile([2 * BL, 1], f32, tag="rs2")
            nc.vector.reciprocal(rs2[:, :], ssum)
            nc.vector.tensor_scalar_mul(wgt2[:, :], ex[:, :], rs2[:, 0:1])

            # history mean: combine the two halves
            nc.scalar.copy(histF[:, :], hist0)
            nc.vector.tensor_add(histF[:, :], histF[:, :], hist1)
            nc.scalar.copy(histTb[:, :], histF[:, :])

            # attention weight rows: (128,100) -> (100,128) -> row layout
            wtp = misc[0:T // 2, 300:364].bitcast(bf16)
            nc.tensor.transpose(wtp, wgt2[:, :], identB128[:, :])
            rows_s = at.tile([T // 2, 2 * BL], bf16, tag="rows_s")
            nc.scalar.copy(rows_s[:, :], wtp)
            nc.sync.dma_start(rowsAll[:, :], rows_s[:, :])

        # ================= AUGRU =========================================
        hA = [zeroT[:, :]]
        with tc.tile_pool(name="axp", bufs=2, space="PSUM") as axp, \
             tc.tile_pool(name="abp", bufs=2, space="PSUM") as abp, \
             tc.tile_pool(name="ut", bufs=5) as ut:
            for c in range(NCG):
                t0 = c * CG
                gc = gruT[:, t0 * BL:(t0 + CG) * BL]
                # slab: [r | u | xh] x CG steps x 64
                slab = axp.tile([D, 3 * CG * BL], f32, tag="ru")
                sl4 = slab.rearrange("p (g s c) -> p g s c", g=3, c=BL)
                _mm(nc, sl4[:, 0, :, :], WrxT, gc, start=True, stop=True)
                _mm(nc, sl4[:, 1, :, :], WuxT, gc, start=True, stop=True,
                    skip=True)
                _mm(nc, sl4[:, 2, :, :], WaxT, gc, start=True, stop=True,
                    skip=True)
                pab = abp.tile([D, CG * BL], f32, tag="pab")
                pab3 = pab.rearrange("p (s c) -> p s c", c=BL)
                _mm(nc, pab[:, :], ones1[:, :],
                    rowsAll[:, t0 * BL:(t0 + CG) * BL], start=True, stop=True)
                if nonzero_bias:
                    nc.vector.tensor_add(
                        sl4[:, 0:2, :, :], sl4[:, 0:2, :, :],
                        bA[:, 0:2].unsqueeze(2).unsqueeze(3)
                        .broadcast_to([D, 2, CG, BL]))
                for s in range(CG):
                    t = t0 + s
                    hprev = hA[0]
                    _mm(nc, sl4[:, 0, s, :], WrhT, hprev,
                        start=False, stop=True, skip=True)
                    _mm(nc, sl4[:, 1, s, :], WuhT, hprev,
                        start=False, stop=True, skip=True)
                    ruS = ut.tile([D, 2 * BL], bf16, tag="ruS")
                    ruS2 = ruS.rearrange("p (g c) -> p g c", g=2)
                    nc.scalar.activation(ruS2[:, :, :], sl4[:, 0:2, s, :],
                                         SIG)
                    hr = ut.tile([D, BL], bf16, tag="hr")
                    nc.vector.tensor_mul(hr[:, :], hprev, ruS2[:, 0, :])
                    _mm(nc, sl4[:, 2, s, :], WahT, hr[:, :],
                        start=False, stop=True, skip=True)
                    # off-path: u' = a*u, 1-u', (1-u')*h
                    up = ut.tile([D, BL], bf16, tag="up")
                    nc.vector.tensor_mul(up[:, :], pab3[:, s, :],
                                         ruS2[:, 1, :])
                    omu = ut.tile([D, BL], bf16, tag="omu")
                    nc.gpsimd.tensor_scalar(omu[:, :], up[:, :], -1.0,
                                            1.0, ALU.mult, ALU.add)
                    omuh = ut.tile([D, BL], bf16, tag="omuh")
                    nc.gpsimd.tensor_mul(omuh[:, :], omu[:, :], hprev)
                    hh = ut.tile([D, BL], bf16, tag="hh")
                    if nonzero_bias:
                        nc.scalar.activation(hh[:, :], sl4[:, 2, s, :],
                                             TANH, bias=bA[:, 2:3])
                    else:
                        nc.scalar.activation(hh[:, :], sl4[:, 2, s, :], TANH)
                    uh = ut.tile([D, BL], bf16, tag="uh")
                    nc.vector.tensor_mul(uh[:, :], up[:, :], hh[:, :])
                    hnew = ut.tile([D, BL], bf16, tag="hA")
                    nc.vector.tensor_add(hnew[:, :], uh[:, :], omuh[:, :])
                    hA[0] = hnew

        # ---------------- output layer -----------------------------------
        with tc.tile_pool(name="ops", bufs=1, space="PSUM") as ops, \
             tc.tile_pool(name="ot", bufs=1) as ot:
            ih = ot.tile([D, BL], bf16)
            nc.vector.tensor_mul(ih[:, :], itemTb[:, :], histTb[:, :])
            po = ops.tile([1, BL], f32)
            pieces = [(userTb[:, :], 0), (itemTb[:, :], 1), (histTb[:, :], 2),
                      (ih[:, :], 3)]
            for piece, g in pieces:
                _mm(nc, po[:, :], outWT[:, g:g + 1], piece,
                    start=(g == 0), stop=False)
            _mm(nc, po[:, :], outWT[:, 4:5], hA[0],
                start=False, stop=True)
            outs = ot.tile([1, BL], f32)
            if nonzero_bias:
                nc.scalar.activation(outs[:, :], po[:, :], AF.Identity,
                                     bias=b2ov[0:1, 0:1])
            else:
                nc.scalar.copy(outs[:, :], po[:, :])
            nc.sync.dma_start(outd[:, :], outs[:, :])

    nc.finalize()
    return nc


_NC = {}


def _get_nc(nonzero_bias=False):
    if nonzero_bias not in _NC:
        _NC[nonzero_bias] = build_nc(nonzero_bias)
    return _NC[nonzero_bias]


def make_in_maps(inputs):
    """Slice + marshal full inputs into per-core input maps (host-side)."""
    f = {k: np.asarray(v) for k, v in inputs.items()}
    nonzero_bias = any(
        np.any(f[k]) for k in ("gru_bih", "gru_bhh", "attn_b0", "attn_b1",
                               "attn_b2", "aug_br", "aug_bu", "aug_bh",
                               "out_b"))

    # gate layout [r | z | -z | n]: the -z columns give omz = sigmoid(-x_z)
    # = 1 - z via the same merged sigmoid, at the cost of one extra matmul.
    Wih_t = f["gru_Wih"].T    # (128, 384) [r|z|n]
    Whh_t = f["gru_Whh"].T
    WihT = np.ascontiguousarray(np.concatenate(
        [Wih_t[:, 0:D], Wih_t[:, D:2 * D], -Wih_t[:, D:2 * D],
         Wih_t[:, 2 * D:3 * D]], axis=1).astype(BF))             # (128, 512)
    WhhT = np.ascontiguousarray(np.concatenate(
        [Whh_t[:, 0:D], Whh_t[:, D:2 * D], -Whh_t[:, D:2 * D],
         Whh_t[:, 2 * D:3 * D]], axis=1).astype(BF))
    W0 = f["attn_W0"]                                            # (80, 512)
    W0f, W0q, W0m, W0d = (W0[:, 0:D], W0[:, D:2 * D],
                          W0[:, 2 * D:3 * D], W0[:, 3 * D:4 * D])
    W0aT = np.ascontiguousarray((W0f - W0d).T.astype(BF))        # (128, 80)
    W0cT = np.ascontiguousarray(W0m.T.astype(BF))
    W0qT = np.ascontiguousarray((W0q + W0d).T.astype(BF))
    selQ = np.ascontiguousarray(np.tile(np.eye(BL, dtype=np.float32),
                                        (1, CH)).astype(BF))     # (64, 512)
    stack2 = np.ascontiguousarray(np.tile(np.eye(BL, dtype=np.float32),
                                          (2, 2)))               # (128, 128)
    W1T = np.ascontiguousarray(f["attn_W1"].T.astype(BF))        # (80, 40)
    W2T = np.ascontiguousarray(f["attn_W2"].T.astype(BF))        # (40, 1)
    augW = np.concatenate(
        [np.ascontiguousarray(f[k][:, p * D:(p + 1) * D].T)
         for k in ("aug_Wr", "aug_Wu", "aug_Wh") for p in (0, 1)],
        axis=1).astype(BF)                                       # (128, 768)
    outWT = np.ascontiguousarray(f["out_W"].reshape(5, D).T.astype(BF))

    shared = dict(WihT=WihT, WhhT=WhhT, W0aT=W0aT, W0cT=W0cT, W0qT=W0qT,
                  selQ=selQ, stack2=stack2, W1T=W1T, W2T=W2T, augW=augW,
                  outWT=outWT)
    if nonzero_bias:
        bih3, bhh3 = f["gru_bih"].reshape(3, D), f["gru_bhh"].reshape(3, D)
        # [br, bz, bhn, bin]: r/z biases merged; n-gate keeps the recurrent
        # part (scaled by r) and the input part (tanh bias) separate.
        bG = np.stack([bih3[0] + bhh3[0], bih3[1] + bhh3[1], bhh3[2],
                       bih3[2]], axis=1).astype(np.float32)      # (128, 4)
        bA = np.stack([f["aug_br"], f["aug_bu"], f["aug_bh"]],
                      axis=1).astype(np.float32)
        shared.update(
            bG=np.ascontiguousarray(bG), bA=np.ascontiguousarray(bA),
            b0=np.ascontiguousarray(f["attn_b0"].reshape(80, 1).astype(np.float32)),
            b1=np.ascontiguousarray(f["attn_b1"].reshape(40, 1).astype(np.float32)),
            b2o=np.ascontiguousarray(f["attn_b2"].reshape(1, 1).astype(np.float32)))

    mask = f["mask"].astype(np.float32)          # (B, T)
    hist = f["item_historical_embedding"]        # (B, T, D)
    in_maps = []
    for ci in range(NCORES):
        s = slice(ci * BL, (ci + 1) * BL)
        m = dict(shared)
        hs = hist[s]                                             # (64,200,128)
        m["histT"] = np.ascontiguousarray(
            hs.transpose(2, 1, 0).reshape(D, T * BL).astype(BF))
        m["histN0"] = np.ascontiguousarray(
            hs[:, 0:TH, :].transpose(1, 0, 2).reshape(TH, BL * D).astype(BF))
        m["histN1"] = np.ascontiguousarray(
            hs[:, TH:T, :].transpose(1, 0, 2).reshape(TL, BL * D).astype(BF))
        msk = mask[s]                                            # (64, 200)
        seq = f["sequential_length"][s].astype(np.float32)       # (64,)
        msc = msk / seq[:, None]                                 # (64, 200)
        m["mstT"] = np.ascontiguousarray(msc[:, 0:TH].T.astype(BF))
        m["mstT1"] = np.ascontiguousarray(msc[:, TH:T].T.astype(BF))
        # additive softmax mask in the (128, 100) 2-step layout:
        # row p<64 -> t even (t=2j, b=p); row p>=64 -> t odd (t=2j+1, b=p-64)
        m2 = np.zeros((2 * BL, T // 2), dtype=np.float32)
        m2[0:BL, :] = np.where(msk[:, 0::2] == 1, 0.0, -1e9)
        m2[BL:2 * BL, :] = np.where(msk[:, 1::2] == 1, 0.0, -1e9)
        m["m2add"] = np.ascontiguousarray(m2)
        m["itemTb"] = np.ascontiguousarray(f["item_embedding"][s].T.astype(BF))
        m["userTb"] = np.ascontiguousarray(f["user_embedding"][s].T.astype(BF))
        in_maps.append(m)
    return in_maps, nonzero_bias


def kernel(**inputs) -> np.ndarray:
    in_maps, nonzero_bias = make_in_maps(inputs)
    nc = _get_nc(nonzero_bias)
    res = run_bass_kernel_spmd(nc, in_maps, list(range(NCORES)))
    return np.concatenate(
        [np.asarray(res.results[c]["out"]).reshape(BL) for c in range(NCORES)])
